# revision 14
# baseline (speedup 1.0000x reference)
"""Contrastive (SimCLR-style) loss on 8 Trainium2 NeuronCores.

Math: with X = concat(z_i, z_j) [8192, 256], x_hat = row-L2-normalized X,
  logits = (x_hat @ x_hat.T) / T          (T = 0.5)
  loss = mean_r [ logsumexp_j(logits[r, j]) - logits[r, label(r)] ]
  label(r) = 2r for r < 4096 else 2(r - 4096) + 1.

Rows are unit-norm so logits are in [-2, 2]: exp() cannot overflow and the
log-softmax max-subtraction can be skipped exactly.

Sharding: row-parallel. Core c owns stacked rows [1024c, 1024(c+1)).  The
full normalized matrix (bf16, transposed to [K=256, 8192]) is replicated to
every core as the matmul moving operand; the core's own 1024 rows are its
stationary operand.  Each core emits per-row partial exp-sums (4 column
groups of 2048) plus the positive-pair dot; the host finishes with
log/sum/mean.  No device collective is needed.

Per-core device pipeline:
  PE:  [K=128,M=128] x [K=128,N=512] bf16 matmuls, K=256 via 2-step PSUM
       accumulation, into [128, 2048] PSUM tiles (4 banks, double buffered)
  ACT: exp(2.0 * psum) -> SBUF bf16 (one instr per 2048 cols)
  DVE: row-sum of the exp tiles (bf16 2x mode) + positive-pair dots

All DRAM tensors are host-pre-permuted to partition-major [128, X] layout so
every DMA is a single fully-contiguous-per-partition transfer.
"""

import os
import sys

if "/opt/trn_rl_repo" not in sys.path:
    sys.path.insert(0, "/opt/trn_rl_repo")

import numpy as np
import ml_dtypes

TEMPERATURE = 0.5
EPS = 1e-12
N = 4096
D = 256
TWO_N = 2 * N
N_CORES = 8
ROWS_PER_CORE = TWO_N // N_CORES  # 1024
M_TILES = ROWS_PER_CORE // 128  # 8
K_CH = D // 128  # 2
N_GROUPS = 4  # psum column groups (2048 cols each)
GROUP_COLS = TWO_N // N_GROUPS  # 2048

_CACHE = {}

last_results = {"exec_time_ns": None, "mean_exec_time_ns": None}

_AXON_PJRT_SO = "/opt/axon/libaxon_pjrt.so"


def _ensure_ntff_hook():
    """Provide antenv.axon_hooks if the image lacks it (profiling glue only).

    The axon PJRT .so exposes axon_{start,stop}_nrt_profile; concourse looks
    the hook up via antenv.axon_hooks which this image doesn't ship. Inject
    an equivalent module so run_bass_kernel_spmd(trace=True) can capture
    NTFF profiles. Also neuter the artifact upload (no bucket here).
    """
    import contextlib
    import ctypes
    import types

    try:
        from antenv.axon_hooks import get_axon_ntff_profile_hook  # noqa: F401

        return
    except ImportError:
        pass

    lib = ctypes.CDLL(_AXON_PJRT_SO)
    lib.axon_start_nrt_profile.argtypes = [
        ctypes.POINTER(ctypes.c_int64),
        ctypes.c_size_t,
    ]
    lib.axon_start_nrt_profile.restype = ctypes.c_int64
    lib.axon_stop_nrt_profile.argtypes = [ctypes.c_char_p]
    lib.axon_stop_nrt_profile.restype = ctypes.c_int64

    @contextlib.contextmanager
    def _hook(output_dir, device_ids):
        import jax

        jax.devices()
        if device_ids:
            ids = (ctypes.c_int64 * len(device_ids))(*device_ids)
            rc = lib.axon_start_nrt_profile(ids, len(device_ids))
        else:
            rc = lib.axon_start_nrt_profile(None, 0)
        if rc != 0:
            raise RuntimeError(f"axon_start_nrt_profile rc={rc}")
        try:
            yield
        finally:
            n = lib.axon_stop_nrt_profile(str(output_dir).encode())
            print(f"ntff profile: {n} file(s) written to {output_dir}", file=sys.stderr)

    mod = types.ModuleType("antenv.axon_hooks")
    mod.get_axon_ntff_profile_hook = lambda: _hook
    mod.set_axon_ntff_profile_hook = lambda h: None
    import antenv

    antenv.axon_hooks = mod
    sys.modules["antenv.axon_hooks"] = mod

    from concourse import bass_utils as _bu

    _bu.upload_artifacts = lambda tmpdir: f"local://{tmpdir}"


def _build_module():
    """Build + compile the (SPMD-identical) single-core bass program."""
    from concourse import bacc, bass, mybir, tile

    f32 = mybir.dt.float32
    bf16 = mybir.dt.bfloat16
    AF = mybir.ActivationFunctionType
    ALU = mybir.AluOpType

    nc = bacc.Bacc(
        "TRN2",
        target_bir_lowering=False,
        debug=False,
        enable_asserts=True,
        num_devices=N_CORES,
    )

    # All inputs partition-major, fully contiguous per DMA.
    # xt[k*4+g] is rhs chunk (K-chunk k, column group g): [128, 2048].
    xt = nc.dram_tensor("xt", [K_CH * N_GROUPS, 128, GROUP_COLS], bf16, kind="ExternalInput")
    # lhsT: [128, 2048] = K-chunks side by side ([,:1024]=k0 cols, [,1024:]=k1)
    lhsT = nc.dram_tensor("lhsT", [128, K_CH * ROWS_PER_CORE], bf16, kind="ExternalInput")
    lloc = nc.dram_tensor("lloc", [128, M_TILES * D], bf16, kind="ExternalInput")
    ppart = nc.dram_tensor("ppart", [128, M_TILES * D], bf16, kind="ExternalInput")
    esum = nc.dram_tensor("esum", [128, M_TILES * 2], f32, kind="ExternalOutput")
    pdot = nc.dram_tensor("pdot", [128, M_TILES], f32, kind="ExternalOutput")

    with tile.TileContext(nc) as tc:
        with (
            tc.tile_pool(name="rhs", bufs=1) as rhs_pool,
            tc.tile_pool(name="small", bufs=1) as small_pool,
            tc.tile_pool(name="scr", bufs=3) as scr_pool,
            tc.tile_pool(name="escr", bufs=6) as escr_pool,
            tc.tile_pool(name="acc", bufs=1) as acc_pool,
            tc.tile_pool(name="ps", bufs=2, space=bass.MemorySpace.PSUM) as psum_pool,
        ):
            # lhsT first: the first matmul needs it.
            lh = small_pool.tile([128, K_CH * ROWS_PER_CORE], bf16, tag="lhsT")
            nc.sync.dma_start(lh[:], lhsT[:])

            # rhs chunks spread across 3 DMA queues (sync HWDGE, gpsimd SWDGE,
            # scalar HWDGE), interleaved so groups arrive in consumption order.
            dma_eng = {
                (0, 0): nc.sync, (1, 0): nc.gpsimd,
                (0, 1): nc.scalar, (1, 1): nc.gpsimd,
                (0, 2): nc.sync, (1, 2): nc.scalar,
                (0, 3): nc.gpsimd, (1, 3): nc.sync,
            }
            chunks = [[None] * N_GROUPS for _ in range(K_CH)]
            for g in range(N_GROUPS):
                for k in range(K_CH):
                    t = rhs_pool.tile([128, GROUP_COLS], bf16, tag=f"xt_{k}_{g}")
                    dma_eng[(k, g)].dma_start(t[:], xt[k * N_GROUPS + g])
                    chunks[k][g] = t

            # positive-pair inputs + dots on DVE (independent path, loads last)
            ll = small_pool.tile([128, M_TILES * D], bf16, tag="lloc")
            pp = small_pool.tile([128, M_TILES * D], bf16, tag="ppart")
            nc.gpsimd.dma_start(ll[:], lloc[:])
            nc.gpsimd.dma_start(pp[:], ppart[:])
            pd = acc_pool.tile([128, M_TILES], f32, tag="pdot")
            for t in range(M_TILES):
                pscr = scr_pool.tile([128, D], f32, tag="pscr")
                nc.vector.tensor_mul(
                    pscr[:], ll[:, t * D : (t + 1) * D], pp[:, t * D : (t + 1) * D]
                )
                nc.vector.tensor_reduce(
                    pd[:, t : t + 1], pscr[:], axis=mybir.AxisListType.X, op=ALU.add
                )
            nc.sync.dma_start(pdot[:], pd[:])

            # main loop: matmul -> exp (ACT) -> CCE-DMA pair-fold -> row-sum (DVE)
            es = acc_pool.tile([128, M_TILES * 2], f32, tag="esum")
            for m in range(M_TILES):
                egs = []
                for g in range(N_GROUPS):
                    ps = psum_pool.tile([128, GROUP_COLS], f32, tag="ps")
                    MM_N = 512  # one PSUM bank per matmul
                    for n in range(GROUP_COLS // MM_N):
                        for k in range(K_CH):
                            nc.tensor.matmul(
                                ps[:, n * MM_N : (n + 1) * MM_N],
                                lh[:, k * ROWS_PER_CORE + m * 128 : k * ROWS_PER_CORE + (m + 1) * 128],
                                chunks[k][g][:, n * MM_N : (n + 1) * MM_N],
                                start=(k == 0),
                                stop=(k == K_CH - 1),
                            )
                    escr = escr_pool.tile([128, GROUP_COLS], bf16, tag="escr")
                    nc.scalar.activation(
                        escr[:],
                        ps[:],
                        AF.Exp,
                        bias=0.0,
                        scale=1.0 / TEMPERATURE,
                    )
                    egs.append(escr)
                    if g % 2 == 1:
                        # fold the pair on the (idle) DMA path: egs[g-1] += egs[g]
                        nc.gpsimd.dma_start(
                            egs[g - 1][:], egs[g][:], accum_op=ALU.add
                        )
                        nc.vector.tensor_reduce(
                            es[:, m * 2 + g // 2 : m * 2 + g // 2 + 1],
                            egs[g - 1][:],
                            axis=mybir.AxisListType.X,
                            op=ALU.add,
                        )
            nc.sync.dma_start(esum[:], es[:])

    nc.compile()
    return nc


def _prep_inputs(z_i, z_j):
    """Host prep: normalize (fp64), cast bf16, build per-core input maps.

    Everything is laid out partition-major to match SBUF tiles exactly.
    """
    X = np.concatenate(
        [np.asarray(z_i, dtype=np.float64), np.asarray(z_j, dtype=np.float64)], axis=0
    )
    norms = np.sqrt((X * X).sum(axis=1, keepdims=True))
    Xn = X / np.maximum(norms, EPS)
    Xn16 = Xn.astype(ml_dtypes.bfloat16)

    XT = np.ascontiguousarray(Xn16.T)  # [D, 2N]
    # xt chunk (k, g): [128, GROUP_COLS]; stored [K_CH*N_GROUPS, 128, GROUP_COLS]
    # XT.reshape(K_CH, 128, N_GROUPS, GROUP_COLS) axes (k, p, g, col)
    xt_np = np.ascontiguousarray(
        XT.reshape(K_CH, 128, N_GROUPS, GROUP_COLS)
        .transpose(0, 2, 1, 3)
        .reshape(K_CH * N_GROUPS, 128, GROUP_COLS)
    )

    labels = np.empty(TWO_N, dtype=np.int64)
    labels[:N] = 2 * np.arange(N)
    labels[N:] = 2 * np.arange(N) + 1

    in_maps = []
    for c in range(N_CORES):
        r0, r1 = c * ROWS_PER_CORE, (c + 1) * ROWS_PER_CORE
        # lhsT [128, K_CH*1024]: col k*1024 + j holds XT[k*128 + p, r0 + j]
        lhsT_np = np.ascontiguousarray(
            XT[:, r0:r1].reshape(K_CH, 128, ROWS_PER_CORE).transpose(1, 0, 2).reshape(128, -1)
        )
        # lloc [128, M_TILES*D]: col t*D + d holds Xn16[r0 + t*128 + p, d]
        lloc_np = np.ascontiguousarray(
            Xn16[r0:r1].reshape(M_TILES, 128, D).transpose(1, 0, 2).reshape(128, -1)
        )
        ppart_np = np.ascontiguousarray(
            Xn16[labels[r0:r1]].reshape(M_TILES, 128, D).transpose(1, 0, 2).reshape(128, -1)
        )
        in_maps.append(
            {"xt": xt_np, "lhsT": lhsT_np, "lloc": lloc_np, "ppart": ppart_np}
        )
    return in_maps


def _finish(results):
    """Host reduction: loss = mean( ln(sum_j exp) - 2*posdot )."""
    total = 0.0
    for c in range(N_CORES):
        # esum [128, M_TILES*N_GROUPS] -> per-row sums over the 4 groups
        es = results[c]["esum"].astype(np.float64).reshape(128, M_TILES, 2)
        S = es.sum(axis=-1)  # [128, M_TILES]
        pd = results[c]["pdot"].astype(np.float64)  # [128, M_TILES]
        total += (np.log(S) - (1.0 / TEMPERATURE) * pd).sum()
    return np.float32(total / TWO_N)


def kernel(z_i, z_j, backend="hw", profile=None):
    """Full inputs in, full output out. backend: "hw" or "sim"."""
    in_maps = _prep_inputs(z_i, z_j)

    if "nc" not in _CACHE:
        _CACHE["nc"] = _build_module()
    nc = _CACHE["nc"]

    if backend == "sim":
        from concourse.bass_interp import CoreSim

        results = []
        for c in range(N_CORES):
            sim = CoreSim(nc)
            for name, val in in_maps[c].items():
                sim.tensor(name)[:] = val
            sim.simulate()
            results.append(
                {
                    "esum": np.array(sim.tensor("esum")),
                    "pdot": np.array(sim.tensor("pdot")),
                }
            )
        return _finish(results)

    from concourse.bass_utils import run_bass_kernel_spmd

    if profile is None:
        profile = bool(int(os.environ.get("KERNEL_PROFILE", "0")))
    if profile:
        _ensure_ntff_hook()
    br = run_bass_kernel_spmd(
        nc, in_maps, core_ids=list(range(N_CORES)), trace=profile
    )
    last_results["exec_time_ns"] = br.exec_time_ns
    last_results["mean_exec_time_ns"] = br.mean_exec_time_ns
    return _finish(br.results)


# revision 15
# speedup vs baseline: 1.0034x; 1.0034x over previous
"""Contrastive (SimCLR-style) loss on 8 Trainium2 NeuronCores.

Math: with X = concat(z_i, z_j) [8192, 256], x_hat = row-L2-normalized X,
  logits = (x_hat @ x_hat.T) / T          (T = 0.5)
  loss = mean_r [ logsumexp_j(logits[r, j]) - logits[r, label(r)] ]
  label(r) = 2r for r < 4096 else 2(r - 4096) + 1.

Rows are unit-norm so logits are in [-2, 2]: exp() cannot overflow and the
log-softmax max-subtraction can be skipped exactly.

Sharding: row-parallel. Core c owns stacked rows [1024c, 1024(c+1)).  The
full normalized matrix (bf16, transposed to [K=256, 8192]) is replicated to
every core as the matmul moving operand; the core's own 1024 rows are its
stationary operand.  Each core emits per-row partial exp-sums (4 column
groups of 2048) plus the positive-pair dot; the host finishes with
log/sum/mean.  No device collective is needed.

Per-core device pipeline:
  PE:  [K=128,M=128] x [K=128,N=512] bf16 matmuls, K=256 via 2-step PSUM
       accumulation, into [128, 2048] PSUM tiles (4 banks, double buffered)
  ACT: exp(2.0 * psum) -> SBUF bf16 (one instr per 2048 cols)
  DVE: row-sum of the exp tiles (bf16 2x mode) + positive-pair dots

All DRAM tensors are host-pre-permuted to partition-major [128, X] layout so
every DMA is a single fully-contiguous-per-partition transfer.
"""

import os
import sys

if "/opt/trn_rl_repo" not in sys.path:
    sys.path.insert(0, "/opt/trn_rl_repo")

import numpy as np
import ml_dtypes

TEMPERATURE = 0.5
EPS = 1e-12
N = 4096
D = 256
TWO_N = 2 * N
N_CORES = 8
ROWS_PER_CORE = TWO_N // N_CORES  # 1024
M_TILES = ROWS_PER_CORE // 128  # 8
K_CH = D // 128  # 2
N_GROUPS = 4  # psum column groups (2048 cols each)
GROUP_COLS = TWO_N // N_GROUPS  # 2048

_CACHE = {}

last_results = {"exec_time_ns": None, "mean_exec_time_ns": None}

_AXON_PJRT_SO = "/opt/axon/libaxon_pjrt.so"


def _ensure_ntff_hook():
    """Provide antenv.axon_hooks if the image lacks it (profiling glue only).

    The axon PJRT .so exposes axon_{start,stop}_nrt_profile; concourse looks
    the hook up via antenv.axon_hooks which this image doesn't ship. Inject
    an equivalent module so run_bass_kernel_spmd(trace=True) can capture
    NTFF profiles. Also neuter the artifact upload (no bucket here).
    """
    import contextlib
    import ctypes
    import types

    try:
        from antenv.axon_hooks import get_axon_ntff_profile_hook  # noqa: F401

        return
    except ImportError:
        pass

    lib = ctypes.CDLL(_AXON_PJRT_SO)
    lib.axon_start_nrt_profile.argtypes = [
        ctypes.POINTER(ctypes.c_int64),
        ctypes.c_size_t,
    ]
    lib.axon_start_nrt_profile.restype = ctypes.c_int64
    lib.axon_stop_nrt_profile.argtypes = [ctypes.c_char_p]
    lib.axon_stop_nrt_profile.restype = ctypes.c_int64

    @contextlib.contextmanager
    def _hook(output_dir, device_ids):
        import jax

        jax.devices()
        if device_ids:
            ids = (ctypes.c_int64 * len(device_ids))(*device_ids)
            rc = lib.axon_start_nrt_profile(ids, len(device_ids))
        else:
            rc = lib.axon_start_nrt_profile(None, 0)
        if rc != 0:
            raise RuntimeError(f"axon_start_nrt_profile rc={rc}")
        try:
            yield
        finally:
            n = lib.axon_stop_nrt_profile(str(output_dir).encode())
            print(f"ntff profile: {n} file(s) written to {output_dir}", file=sys.stderr)

    mod = types.ModuleType("antenv.axon_hooks")
    mod.get_axon_ntff_profile_hook = lambda: _hook
    mod.set_axon_ntff_profile_hook = lambda h: None
    import antenv

    antenv.axon_hooks = mod
    sys.modules["antenv.axon_hooks"] = mod

    from concourse import bass_utils as _bu

    _bu.upload_artifacts = lambda tmpdir: f"local://{tmpdir}"


def _build_module():
    """Build + compile the (SPMD-identical) single-core bass program."""
    from concourse import bacc, bass, mybir, tile

    f32 = mybir.dt.float32
    bf16 = mybir.dt.bfloat16
    AF = mybir.ActivationFunctionType
    ALU = mybir.AluOpType

    nc = bacc.Bacc(
        "TRN2",
        target_bir_lowering=False,
        debug=False,
        enable_asserts=True,
        num_devices=N_CORES,
    )

    # All inputs partition-major, fully contiguous per DMA.
    # xt[k*4+g] is rhs chunk (K-chunk k, column group g): [128, 2048].
    xt = nc.dram_tensor("xt", [K_CH * N_GROUPS, 128, GROUP_COLS], bf16, kind="ExternalInput")
    # lhsT: [128, 2048] = K-chunks side by side ([,:1024]=k0 cols, [,1024:]=k1)
    lhsT = nc.dram_tensor("lhsT", [128, K_CH * ROWS_PER_CORE], bf16, kind="ExternalInput")
    lloc = nc.dram_tensor("lloc", [128, M_TILES * D], bf16, kind="ExternalInput")
    ppart = nc.dram_tensor("ppart", [128, M_TILES * D], bf16, kind="ExternalInput")
    esum = nc.dram_tensor("esum", [128, M_TILES * 2 + 2], f32, kind="ExternalOutput")
    pdot = nc.dram_tensor("pdot", [128, M_TILES], f32, kind="ExternalOutput")

    with tile.TileContext(nc) as tc:
        with (
            tc.tile_pool(name="rhs", bufs=1) as rhs_pool,
            tc.tile_pool(name="small", bufs=1) as small_pool,
            tc.tile_pool(name="scr", bufs=3) as scr_pool,
            tc.tile_pool(name="escr", bufs=6) as escr_pool,
            tc.tile_pool(name="acc", bufs=1) as acc_pool,
            tc.tile_pool(name="ps", bufs=2, space=bass.MemorySpace.PSUM) as psum_pool,
        ):
            # lhsT m-major: col m*256 + k*128 + j. Head (m=0 slice) loads
            # first so the first matmul group is gated by only 64 KB.
            lh = small_pool.tile([128, K_CH * ROWS_PER_CORE], bf16, tag="lhsT")
            nc.sync.dma_start(lh[:, 0 : K_CH * 128], lhsT[:, 0 : K_CH * 128])

            # rhs chunks spread across 3 DMA queues (sync HWDGE, gpsimd SWDGE,
            # scalar HWDGE), interleaved so groups arrive in consumption order.
            dma_eng = {
                (0, 0): nc.sync, (1, 0): nc.gpsimd,
                (0, 1): nc.scalar, (1, 1): nc.gpsimd,
                (0, 2): nc.sync, (1, 2): nc.scalar,
                (0, 3): nc.scalar, (1, 3): nc.gpsimd,
            }
            chunks = [[None] * N_GROUPS for _ in range(K_CH)]
            for g in range(N_GROUPS):
                for k in range(K_CH):
                    t = rhs_pool.tile([128, GROUP_COLS], bf16, tag=f"xt_{k}_{g}")
                    dma_eng[(k, g)].dma_start(t[:], xt[k * N_GROUPS + g])
                    chunks[k][g] = t
            # rest of lhsT (m=1..7) after the early chunks on the sync queue
            nc.sync.dma_start(
                lh[:, K_CH * 128 :], lhsT[:, K_CH * 128 :]
            )

            # positive-pair inputs + dots on DVE (independent path, loads last)
            ll = small_pool.tile([128, M_TILES * D], bf16, tag="lloc")
            pp = small_pool.tile([128, M_TILES * D], bf16, tag="ppart")
            nc.gpsimd.dma_start(ll[:], lloc[:])
            nc.gpsimd.dma_start(pp[:], ppart[:])
            pd = acc_pool.tile([128, M_TILES], f32, tag="pdot")
            for t in range(M_TILES):
                pscr = scr_pool.tile([128, D], f32, tag="pscr")
                nc.vector.tensor_mul(
                    pscr[:], ll[:, t * D : (t + 1) * D], pp[:, t * D : (t + 1) * D]
                )
                nc.vector.tensor_reduce(
                    pd[:, t : t + 1], pscr[:], axis=mybir.AxisListType.X, op=ALU.add
                )
            nc.sync.dma_start(pdot[:], pd[:])

            # main loop: matmul -> exp (ACT) -> CCE-DMA pair-fold -> row-sum (DVE)
            es = acc_pool.tile([128, M_TILES * 2 + 2], f32, tag="esum")
            for m in range(M_TILES):
                egs = []
                for g in range(N_GROUPS):
                    ps = psum_pool.tile([128, GROUP_COLS], f32, tag="ps")
                    MM_N = 512  # one PSUM bank per matmul
                    for n in range(GROUP_COLS // MM_N):
                        for k in range(K_CH):
                            nc.tensor.matmul(
                                ps[:, n * MM_N : (n + 1) * MM_N],
                                lh[:, m * K_CH * 128 + k * 128 : m * K_CH * 128 + (k + 1) * 128],
                                chunks[k][g][:, n * MM_N : (n + 1) * MM_N],
                                start=(k == 0),
                                stop=(k == K_CH - 1),
                            )
                    escr = escr_pool.tile([128, GROUP_COLS], bf16, tag="escr")
                    nc.scalar.activation(
                        escr[:],
                        ps[:],
                        AF.Exp,
                        bias=0.0,
                        scale=1.0 / TEMPERATURE,
                    )
                    egs.append(escr)
                    if m == M_TILES - 1:
                        # last row-tile: reduce each group directly so the
                        # kernel tail isn't waiting on a CCE fold chain
                        nc.vector.tensor_reduce(
                            es[:, 2 * M_TILES - 2 + g : 2 * M_TILES - 1 + g],
                            escr[:],
                            axis=mybir.AxisListType.X,
                            op=ALU.add,
                        )
                    elif g % 2 == 1:
                        # fold the pair on the (idle) DMA path: egs[g-1] += egs[g]
                        nc.gpsimd.dma_start(
                            egs[g - 1][:], egs[g][:], accum_op=ALU.add
                        )
                        nc.vector.tensor_reduce(
                            es[:, m * 2 + g // 2 : m * 2 + g // 2 + 1],
                            egs[g - 1][:],
                            axis=mybir.AxisListType.X,
                            op=ALU.add,
                        )
            nc.sync.dma_start(esum[:], es[:])

    nc.compile()
    return nc


def _prep_inputs(z_i, z_j):
    """Host prep: normalize (fp64), cast bf16, build per-core input maps.

    Everything is laid out partition-major to match SBUF tiles exactly.
    """
    X = np.concatenate(
        [np.asarray(z_i, dtype=np.float64), np.asarray(z_j, dtype=np.float64)], axis=0
    )
    norms = np.sqrt((X * X).sum(axis=1, keepdims=True))
    Xn = X / np.maximum(norms, EPS)
    Xn16 = Xn.astype(ml_dtypes.bfloat16)

    XT = np.ascontiguousarray(Xn16.T)  # [D, 2N]
    # xt chunk (k, g): [128, GROUP_COLS]; stored [K_CH*N_GROUPS, 128, GROUP_COLS]
    # XT.reshape(K_CH, 128, N_GROUPS, GROUP_COLS) axes (k, p, g, col)
    xt_np = np.ascontiguousarray(
        XT.reshape(K_CH, 128, N_GROUPS, GROUP_COLS)
        .transpose(0, 2, 1, 3)
        .reshape(K_CH * N_GROUPS, 128, GROUP_COLS)
    )

    labels = np.empty(TWO_N, dtype=np.int64)
    labels[:N] = 2 * np.arange(N)
    labels[N:] = 2 * np.arange(N) + 1

    in_maps = []
    for c in range(N_CORES):
        r0, r1 = c * ROWS_PER_CORE, (c + 1) * ROWS_PER_CORE
        # lhsT [128, K_CH*1024]: col k*1024 + j holds XT[k*128 + p, r0 + j]
        # m-major: col m*(K_CH*128) + k*128 + j  <-  XT[k*128+p, r0+m*128+j]
        lhsT_np = np.ascontiguousarray(
            XT[:, r0:r1]
            .reshape(K_CH, 128, M_TILES, 128)
            .transpose(1, 2, 0, 3)
            .reshape(128, -1)
        )
        # lloc [128, M_TILES*D]: col t*D + d holds Xn16[r0 + t*128 + p, d]
        lloc_np = np.ascontiguousarray(
            Xn16[r0:r1].reshape(M_TILES, 128, D).transpose(1, 0, 2).reshape(128, -1)
        )
        ppart_np = np.ascontiguousarray(
            Xn16[labels[r0:r1]].reshape(M_TILES, 128, D).transpose(1, 0, 2).reshape(128, -1)
        )
        in_maps.append(
            {"xt": xt_np, "lhsT": lhsT_np, "lloc": lloc_np, "ppart": ppart_np}
        )
    return in_maps


def _finish(results):
    """Host reduction: loss = mean( ln(sum_j exp) - 2*posdot )."""
    total = 0.0
    for c in range(N_CORES):
        # esum [128, M_TILES*N_GROUPS] -> per-row sums over the 4 groups
        esr = results[c]["esum"].astype(np.float64)  # [128, 18]
        S = np.empty((128, M_TILES))
        S[:, : M_TILES - 1] = (
            esr[:, : 2 * M_TILES - 2].reshape(128, M_TILES - 1, 2).sum(axis=-1)
        )
        S[:, M_TILES - 1] = esr[:, 2 * M_TILES - 2 :].sum(axis=-1)
        pd = results[c]["pdot"].astype(np.float64)  # [128, M_TILES]
        total += (np.log(S) - (1.0 / TEMPERATURE) * pd).sum()
    return np.float32(total / TWO_N)


def kernel(z_i, z_j, backend="hw", profile=None):
    """Full inputs in, full output out. backend: "hw" or "sim"."""
    in_maps = _prep_inputs(z_i, z_j)

    if "nc" not in _CACHE:
        _CACHE["nc"] = _build_module()
    nc = _CACHE["nc"]

    if backend == "sim":
        from concourse.bass_interp import CoreSim

        results = []
        for c in range(N_CORES):
            sim = CoreSim(nc)
            for name, val in in_maps[c].items():
                sim.tensor(name)[:] = val
            sim.simulate()
            results.append(
                {
                    "esum": np.array(sim.tensor("esum")),
                    "pdot": np.array(sim.tensor("pdot")),
                }
            )
        return _finish(results)

    from concourse.bass_utils import run_bass_kernel_spmd

    if profile is None:
        profile = bool(int(os.environ.get("KERNEL_PROFILE", "0")))
    if profile:
        _ensure_ntff_hook()
    br = run_bass_kernel_spmd(
        nc, in_maps, core_ids=list(range(N_CORES)), trace=profile
    )
    last_results["exec_time_ns"] = br.exec_time_ns
    last_results["mean_exec_time_ns"] = br.mean_exec_time_ns
    return _finish(br.results)


# revision 16
# speedup vs baseline: 1.0877x; 1.0840x over previous
"""Contrastive (SimCLR-style) loss on 8 Trainium2 NeuronCores.

Math: with X = concat(z_i, z_j) [8192, 256], x_hat = row-L2-normalized X,
  logits = (x_hat @ x_hat.T) / T          (T = 0.5)
  loss = mean_r [ logsumexp_j(logits[r, j]) - logits[r, label(r)] ]
  label(r) = 2r for r < 4096 else 2(r - 4096) + 1.

Rows are unit-norm so logits are in [-2, 2]: exp() cannot overflow and the
log-softmax max-subtraction can be skipped exactly.

Sharding: row-parallel. Core c owns stacked rows [1024c, 1024(c+1)).  The
full normalized matrix (bf16, transposed to [K=256, 8192]) is replicated to
every core as the matmul moving operand; the core's own 1024 rows are its
stationary operand.  Each core emits per-row partial exp-sums (4 column
groups of 2048) plus the positive-pair dot; the host finishes with
log/sum/mean.  No device collective is needed.

Per-core device pipeline:
  PE:  [K=128,M=128] x [K=128,N=512] bf16 matmuls, K=256 via 2-step PSUM
       accumulation, into [128, 2048] PSUM tiles (4 banks, double buffered)
  ACT: exp(2.0 * psum) -> SBUF bf16 (one instr per 2048 cols)
  DVE: row-sum of the exp tiles (bf16 2x mode) + positive-pair dots

All DRAM tensors are host-pre-permuted to partition-major [128, X] layout so
every DMA is a single fully-contiguous-per-partition transfer.
"""

import os
import sys

if "/opt/trn_rl_repo" not in sys.path:
    sys.path.insert(0, "/opt/trn_rl_repo")

import numpy as np
import ml_dtypes

TEMPERATURE = 0.5
EPS = 1e-12
N = 4096
D = 256
TWO_N = 2 * N
N_CORES = 8
ROWS_PER_CORE = TWO_N // N_CORES  # 1024
M_TILES = ROWS_PER_CORE // 128  # 8
K_CH = D // 128  # 2
N_GROUPS = 4  # psum column groups (2048 cols each)
GROUP_COLS = TWO_N // N_GROUPS  # 2048

_CACHE = {}

last_results = {"exec_time_ns": None, "mean_exec_time_ns": None}

_AXON_PJRT_SO = "/opt/axon/libaxon_pjrt.so"


def _ensure_ntff_hook():
    """Provide antenv.axon_hooks if the image lacks it (profiling glue only).

    The axon PJRT .so exposes axon_{start,stop}_nrt_profile; concourse looks
    the hook up via antenv.axon_hooks which this image doesn't ship. Inject
    an equivalent module so run_bass_kernel_spmd(trace=True) can capture
    NTFF profiles. Also neuter the artifact upload (no bucket here).
    """
    import contextlib
    import ctypes
    import types

    try:
        from antenv.axon_hooks import get_axon_ntff_profile_hook  # noqa: F401

        return
    except ImportError:
        pass

    lib = ctypes.CDLL(_AXON_PJRT_SO)
    lib.axon_start_nrt_profile.argtypes = [
        ctypes.POINTER(ctypes.c_int64),
        ctypes.c_size_t,
    ]
    lib.axon_start_nrt_profile.restype = ctypes.c_int64
    lib.axon_stop_nrt_profile.argtypes = [ctypes.c_char_p]
    lib.axon_stop_nrt_profile.restype = ctypes.c_int64

    @contextlib.contextmanager
    def _hook(output_dir, device_ids):
        import jax

        jax.devices()
        if device_ids:
            ids = (ctypes.c_int64 * len(device_ids))(*device_ids)
            rc = lib.axon_start_nrt_profile(ids, len(device_ids))
        else:
            rc = lib.axon_start_nrt_profile(None, 0)
        if rc != 0:
            raise RuntimeError(f"axon_start_nrt_profile rc={rc}")
        try:
            yield
        finally:
            n = lib.axon_stop_nrt_profile(str(output_dir).encode())
            print(f"ntff profile: {n} file(s) written to {output_dir}", file=sys.stderr)

    mod = types.ModuleType("antenv.axon_hooks")
    mod.get_axon_ntff_profile_hook = lambda: _hook
    mod.set_axon_ntff_profile_hook = lambda h: None
    import antenv

    antenv.axon_hooks = mod
    sys.modules["antenv.axon_hooks"] = mod

    from concourse import bass_utils as _bu

    _bu.upload_artifacts = lambda tmpdir: f"local://{tmpdir}"


def _build_module():
    """Build + compile the (SPMD-identical) single-core bass program."""
    from concourse import bacc, bass, mybir, tile

    f32 = mybir.dt.float32
    bf16 = mybir.dt.bfloat16
    AF = mybir.ActivationFunctionType
    ALU = mybir.AluOpType

    nc = bacc.Bacc(
        "TRN2",
        target_bir_lowering=False,
        debug=False,
        enable_asserts=True,
        num_devices=N_CORES,
    )

    # All inputs partition-major, fully contiguous per DMA.
    # xt[k*4+g] is rhs chunk (K-chunk k, column group g): [128, 2048].
    xt = nc.dram_tensor("xt", [K_CH * N_GROUPS, 128, GROUP_COLS], bf16, kind="ExternalInput")
    # lhsT: [128, 2048] = K-chunks side by side ([,:1024]=k0 cols, [,1024:]=k1)
    lhsT = nc.dram_tensor("lhsT", [128, K_CH * ROWS_PER_CORE], bf16, kind="ExternalInput")
    lloc = nc.dram_tensor("lloc", [128, M_TILES * D], bf16, kind="ExternalInput")
    ppart = nc.dram_tensor("ppart", [128, M_TILES * D], bf16, kind="ExternalInput")
    esum = nc.dram_tensor("esum", [128, M_TILES * 2 + 2], f32, kind="ExternalOutput")
    pdot = nc.dram_tensor("pdot", [128, M_TILES], f32, kind="ExternalOutput")

    with tile.TileContext(nc) as tc:
        with (
            tc.tile_pool(name="rhs", bufs=1) as rhs_pool,
            tc.tile_pool(name="small", bufs=1) as small_pool,
            tc.tile_pool(name="scr", bufs=3) as scr_pool,
            tc.tile_pool(name="escr", bufs=6) as escr_pool,
            tc.tile_pool(name="acc", bufs=1) as acc_pool,
            tc.tile_pool(name="ps", bufs=2, space=bass.MemorySpace.PSUM) as psum_pool,
        ):
            # lhsT m-major: col m*256 + k*128 + j. Head (m=0 slice) loads
            # first so the first matmul group is gated by only 64 KB.
            lh = small_pool.tile([128, K_CH * ROWS_PER_CORE], bf16, tag="lhsT")
            nc.sync.dma_start(lh[:, 0 : K_CH * 128], lhsT[:, 0 : K_CH * 128])

            # rhs chunks spread across 3 DMA queues (sync HWDGE, gpsimd SWDGE,
            # scalar HWDGE), interleaved so groups arrive in consumption order.
            dma_eng = {
                (0, 0): nc.sync, (1, 0): nc.gpsimd,
                (0, 1): nc.scalar, (1, 1): nc.gpsimd,
                (0, 2): nc.sync, (1, 2): nc.scalar,
                (0, 3): nc.scalar, (1, 3): nc.gpsimd,
            }
            chunks = [[None] * N_GROUPS for _ in range(K_CH)]
            for g in range(N_GROUPS):
                for k in range(K_CH):
                    t = rhs_pool.tile([128, GROUP_COLS], bf16, tag=f"xt_{k}_{g}")
                    dma_eng[(k, g)].dma_start(t[:], xt[k * N_GROUPS + g])
                    chunks[k][g] = t
            # rest of lhsT (m=1..7) after the early chunks on the sync queue
            nc.sync.dma_start(
                lh[:, K_CH * 128 :], lhsT[:, K_CH * 128 :]
            )

            # positive-pair inputs + dots on DVE (independent path, loads last)
            ll = small_pool.tile([128, M_TILES * D], bf16, tag="lloc")
            pp = small_pool.tile([128, M_TILES * D], bf16, tag="ppart")
            nc.gpsimd.dma_start(ll[:], lloc[:])
            nc.gpsimd.dma_start(pp[:], ppart[:])
            pd = acc_pool.tile([128, M_TILES], f32, tag="pdot")
            for t in range(M_TILES):
                pscr = scr_pool.tile([128, D], f32, tag="pscr")
                nc.vector.tensor_mul(
                    pscr[:], ll[:, t * D : (t + 1) * D], pp[:, t * D : (t + 1) * D]
                )
                nc.vector.tensor_reduce(
                    pd[:, t : t + 1], pscr[:], axis=mybir.AxisListType.X, op=ALU.add
                )
            nc.sync.dma_start(pdot[:], pd[:])

            # main loop: matmul -> exp (ACT) -> CCE-DMA pair-fold -> row-sum (DVE)
            es = acc_pool.tile([128, M_TILES * 2 + 2], f32, tag="esum")
            for m in range(M_TILES):
                egs = []
                for g in range(N_GROUPS):
                    ps = psum_pool.tile([128, GROUP_COLS], f32, tag="ps")
                    MM_N = 512  # one PSUM bank per matmul
                    for n in range(GROUP_COLS // MM_N):
                        for k in range(K_CH):
                            nc.tensor.matmul(
                                ps[:, n * MM_N : (n + 1) * MM_N],
                                lh[:, m * K_CH * 128 + k * 128 : m * K_CH * 128 + (k + 1) * 128],
                                chunks[k][g][:, n * MM_N : (n + 1) * MM_N],
                                start=(k == 0),
                                stop=(k == K_CH - 1),
                            )
                    escr = escr_pool.tile([128, GROUP_COLS], bf16, tag="escr")
                    if m == M_TILES - 1:
                        # last row-tile: fused ACT accumulate so the kernel
                        # tail isn't a serial chain of DVE reduces
                        nc.scalar.activation(
                            escr[:],
                            ps[:],
                            AF.Exp,
                            bias=0.0,
                            scale=1.0 / TEMPERATURE,
                            accum_out=es[:, 2 * M_TILES - 2 + g : 2 * M_TILES - 1 + g],
                        )
                        continue
                    nc.scalar.activation(
                        escr[:],
                        ps[:],
                        AF.Exp,
                        bias=0.0,
                        scale=1.0 / TEMPERATURE,
                    )
                    egs.append(escr)
                    if g % 2 == 1:
                        # fold the pair on the (idle) DMA path: egs[g-1] += egs[g]
                        nc.gpsimd.dma_start(
                            egs[g - 1][:], egs[g][:], accum_op=ALU.add
                        )
                        nc.vector.tensor_reduce(
                            es[:, m * 2 + g // 2 : m * 2 + g // 2 + 1],
                            egs[g - 1][:],
                            axis=mybir.AxisListType.X,
                            op=ALU.add,
                        )
            nc.sync.dma_start(esum[:], es[:])

    nc.compile()
    return nc


def _prep_inputs(z_i, z_j):
    """Host prep: normalize (fp64), cast bf16, build per-core input maps.

    Everything is laid out partition-major to match SBUF tiles exactly.
    """
    X = np.concatenate(
        [np.asarray(z_i, dtype=np.float64), np.asarray(z_j, dtype=np.float64)], axis=0
    )
    norms = np.sqrt((X * X).sum(axis=1, keepdims=True))
    Xn = X / np.maximum(norms, EPS)
    Xn16 = Xn.astype(ml_dtypes.bfloat16)

    XT = np.ascontiguousarray(Xn16.T)  # [D, 2N]
    # xt chunk (k, g): [128, GROUP_COLS]; stored [K_CH*N_GROUPS, 128, GROUP_COLS]
    # XT.reshape(K_CH, 128, N_GROUPS, GROUP_COLS) axes (k, p, g, col)
    xt_np = np.ascontiguousarray(
        XT.reshape(K_CH, 128, N_GROUPS, GROUP_COLS)
        .transpose(0, 2, 1, 3)
        .reshape(K_CH * N_GROUPS, 128, GROUP_COLS)
    )

    labels = np.empty(TWO_N, dtype=np.int64)
    labels[:N] = 2 * np.arange(N)
    labels[N:] = 2 * np.arange(N) + 1

    in_maps = []
    for c in range(N_CORES):
        r0, r1 = c * ROWS_PER_CORE, (c + 1) * ROWS_PER_CORE
        # lhsT [128, K_CH*1024]: col k*1024 + j holds XT[k*128 + p, r0 + j]
        # m-major: col m*(K_CH*128) + k*128 + j  <-  XT[k*128+p, r0+m*128+j]
        lhsT_np = np.ascontiguousarray(
            XT[:, r0:r1]
            .reshape(K_CH, 128, M_TILES, 128)
            .transpose(1, 2, 0, 3)
            .reshape(128, -1)
        )
        # lloc [128, M_TILES*D]: col t*D + d holds Xn16[r0 + t*128 + p, d]
        lloc_np = np.ascontiguousarray(
            Xn16[r0:r1].reshape(M_TILES, 128, D).transpose(1, 0, 2).reshape(128, -1)
        )
        ppart_np = np.ascontiguousarray(
            Xn16[labels[r0:r1]].reshape(M_TILES, 128, D).transpose(1, 0, 2).reshape(128, -1)
        )
        in_maps.append(
            {"xt": xt_np, "lhsT": lhsT_np, "lloc": lloc_np, "ppart": ppart_np}
        )
    return in_maps


def _finish(results):
    """Host reduction: loss = mean( ln(sum_j exp) - 2*posdot )."""
    total = 0.0
    for c in range(N_CORES):
        # esum [128, M_TILES*N_GROUPS] -> per-row sums over the 4 groups
        esr = results[c]["esum"].astype(np.float64)  # [128, 18]
        S = np.empty((128, M_TILES))
        S[:, : M_TILES - 1] = (
            esr[:, : 2 * M_TILES - 2].reshape(128, M_TILES - 1, 2).sum(axis=-1)
        )
        S[:, M_TILES - 1] = esr[:, 2 * M_TILES - 2 :].sum(axis=-1)
        pd = results[c]["pdot"].astype(np.float64)  # [128, M_TILES]
        total += (np.log(S) - (1.0 / TEMPERATURE) * pd).sum()
    return np.float32(total / TWO_N)


def kernel(z_i, z_j, backend="hw", profile=None):
    """Full inputs in, full output out. backend: "hw" or "sim"."""
    in_maps = _prep_inputs(z_i, z_j)

    if "nc" not in _CACHE:
        _CACHE["nc"] = _build_module()
    nc = _CACHE["nc"]

    if backend == "sim":
        from concourse.bass_interp import CoreSim

        results = []
        for c in range(N_CORES):
            sim = CoreSim(nc)
            for name, val in in_maps[c].items():
                sim.tensor(name)[:] = val
            sim.simulate()
            results.append(
                {
                    "esum": np.array(sim.tensor("esum")),
                    "pdot": np.array(sim.tensor("pdot")),
                }
            )
        return _finish(results)

    from concourse.bass_utils import run_bass_kernel_spmd

    if profile is None:
        profile = bool(int(os.environ.get("KERNEL_PROFILE", "0")))
    if profile:
        _ensure_ntff_hook()
    br = run_bass_kernel_spmd(
        nc, in_maps, core_ids=list(range(N_CORES)), trace=profile
    )
    last_results["exec_time_ns"] = br.exec_time_ns
    last_results["mean_exec_time_ns"] = br.mean_exec_time_ns
    return _finish(br.results)
